# revision 48
# baseline (speedup 1.0000x reference)
import os
import sys

sys.path.insert(0, "/opt/trn_rl_repo")

import numpy as np

from concourse import mybir, bass, bacc, tile, bass_utils
from concourse.masks import make_identity

T = 1024
D = 512
H = 8
DH = 64
P2 = 2 * T - 1  # 2047
SCALE = 0.125
EB = -4.0  # multiplicative exp-split offset: exp(S*c+EB)*exp(S*p+EB) = e^{2EB} exp(S*(c+p))
BAND = 1151
PDSZ = T * P2 + 4096

FP32 = mybir.dt.float32
F32R = mybir.dt.float32r
FP16 = mybir.dt.float16
EXP = mybir.ActivationFunctionType.Exp
CPY = mybir.ActivationFunctionType.Copy
MUL = mybir.AluOpType.mult


def _r(ap):
    return ap.bitcast(F32R)


def _build():
    nc = bacc.Bacc()
    x_d = nc.declare_dram_parameter("x", (T, D), FP32, isOutput=False)
    pe_d = nc.declare_dram_parameter("pos_enc", (P2, D), FP32, isOutput=False)
    w_d = {}
    for nm in ("Wq", "Wk", "Wv", "Wpos", "Wo"):
        w_d[nm] = nc.declare_dram_parameter(nm, (D, D), FP32, isOutput=False)
    b_d = {}
    for nm in ("bk", "bv", "bo"):
        b_d[nm] = nc.declare_dram_parameter(nm, (D,), FP32, isOutput=False)
    pbu_d = nc.declare_dram_parameter("pbu", (D,), FP32, isOutput=False)
    pbv_d = nc.declare_dram_parameter("pbv", (D,), FP32, isOutput=False)
    ones_d = nc.declare_dram_parameter("onesv", (128,), FP32, isOutput=False)
    out_d = nc.declare_dram_parameter("out", (T, D), FP32, isOutput=True)

    with tile.TileContext(nc) as tc:
        with tc.tile_pool(name="persist", bufs=1) as pers, \
             tc.tile_pool(name="psum", bufs=1, space="PSUM") as psp, \
             tc.tile_pool(name="dram", bufs=1, space="DRAM") as drp:

            def mmt():
                return psp.tile([128, 512], FP32, tag="mm", bufs=6, name="mmt")

            ident = pers.tile([128, 128], FP32, tag="ident", name="ident")
            make_identity(nc, ident)
            ones = pers.tile([128, 128], FP32, tag="ones", name="ones")
            nc.sync.dma_start(out=_r(ones[0:1, 0:128]),
                              in_=_r(ones_d[0:128].rearrange("(o f) -> o f", o=1)))
            nc.sync.dma_start(out=_r(ones[64:65, 0:64]),
                              in_=_r(ones_d[0:64].rearrange("(o f) -> o f", o=1)))
            ebias = pers.tile([128, 1], FP32, tag="ebias", name="ebias")
            nc.vector.memset(ebias, EB)

            bk_sb = pers.tile([128, 4], FP32, tag="bk", name="bk_sb")
            bo_sb = pers.tile([128, 4], FP32, tag="bo", name="bo_sb")
            bv_sb = pers.tile([1, 512], FP32, tag="bv", name="bv_sb")
            nc.sync.dma_start(out=_r(bv_sb[:, :]),
                              in_=_r(b_d["bv"][0:D].rearrange("(o f) -> o f", o=1)))
            for c in range(4):
                nc.sync.dma_start(out=bk_sb[:, c:c + 1],
                                  in_=b_d["bk"][c * 128:(c + 1) * 128].rearrange("(p o) -> p o", o=1))
                nc.sync.dma_start(out=bo_sb[:, c:c + 1],
                                  in_=b_d["bo"][c * 128:(c + 1) * 128].rearrange("(p o) -> p o", o=1))
            pbu_sb = pers.tile([128, 4], FP32, tag="pbu", name="pbu_sb")
            pbv_sb = pers.tile([128, 4], FP32, tag="pbv", name="pbv_sb")
            for c in range(4):
                nc.sync.dma_start(out=pbu_sb[:, c:c + 1],
                                  in_=pbu_d[c * 128:(c + 1) * 128].rearrange("(p o) -> p o", o=1))
                nc.sync.dma_start(out=pbv_sb[:, c:c + 1],
                                  in_=pbv_d[c * 128:(c + 1) * 128].rearrange("(p o) -> p o", o=1))

            # Persistent transposed activations: [p=feature%128, c*1024 + t]
            qU = pers.tile([128, 4096], FP32, tag="qU", name="qU")
            qV = pers.tile([128, 4096], FP32, tag="qV", name="qV")
            kT = pers.tile([128, 4096], FP32, tag="kT", name="kT")
            # v natural with ones column: [p=t%128, sb*520 + h*65 + e], e==64 -> 1.0
            v65 = pers.tile([128, 4160], FP16, tag="v65", name="v65")
            nc.vector.memset(v65.rearrange("p (b e) -> p b e", e=65)[:, :, 64:65], 1.0)
            # pT[p, c*2047 + n] = P[n, c*128+p],  P = pos_enc @ Wpos
            pT = pers.tile([128, 8188], FP32, tag="pT", name="pT")
            # wo_sb[p, h*512 + dout] = Wo[h*64 + p, dout]  (all heads at base partition 0)
            wo_sb = pers.tile([64, 4096], FP32, tag="wo", name="wo_sb")
            outT = pers.tile([64, 8192], FP32, tag="outT", name="outT")

            pd = [drp.tile([PDSZ], FP16, tag=f"pd{h}", name=f"pd{h}") for h in range(H)]

            sX_cm = tc.tile_pool(name="sX", bufs=1)
            sX = sX_cm.__enter__()
            xT = sX.tile([128, 4096], FP32, tag="xT", name="xT")

            # ---- Phase 1: xT via PE transposes ----
            with tc.tile_pool(name="sA", bufs=1) as sA:
                for tb in range(8):
                    xin = sA.tile([128, 512], FP32, tag="xin", bufs=2, name="xin")
                    nc.sync.dma_start(out=xin, in_=x_d[tb * 128:(tb + 1) * 128, :])
                    for c in range(4):
                        pt = mmt()
                        nc.tensor.transpose(pt[:, 0:128], xin[:, c * 128:(c + 1) * 128], ident)
                        nc.vector.tensor_copy(
                            _r(xT[:, c * 1024 + tb * 128: c * 1024 + tb * 128 + 128]),
                            pt[:, 0:128])

            # ---- Phase 2: peT, then pT = Wpos^T-layout matmul ----
            with tc.tile_pool(name="sB", bufs=1) as sB:
                peT = sB.tile([128, 8188], FP32, tag="peT", name="peT")
                wpos = sB.tile([128, 2048], FP32, tag="wpos", name="wpos")
                for c in range(4):
                    nc.sync.dma_start(out=_r(wpos[:, c * 512:(c + 1) * 512]),
                                      in_=_r(w_d["Wpos"][c * 128:(c + 1) * 128, :]))
                for r in range(16):
                    r0 = r * 128
                    nr = min(128, P2 - r0)
                    pin = sB.tile([128, 512], FP32, tag="pin", bufs=2, name="pin")
                    nc.sync.dma_start(out=pin[0:nr, :], in_=pe_d[r0:r0 + nr, :])
                    for c in range(4):
                        pt = mmt()
                        nc.tensor.transpose(pt[:, 0:nr], pin[0:nr, c * 128:(c + 1) * 128],
                                            ident[0:nr, 0:nr])
                        nc.vector.tensor_copy(_r(peT[:, c * P2 + r0: c * P2 + r0 + nr]),
                                              pt[:, 0:nr])
                for m in range(4):
                    for (n0, nn) in ((0, 512), (512, 512), (1024, 512), (1535, 512)):
                        acc = mmt()
                        for c in range(4):
                            nc.tensor.matmul(acc[:, 0:nn],
                                             _r(wpos[:, c * 512 + m * 128: c * 512 + m * 128 + 128]),
                                             _r(peT[:, c * P2 + n0: c * P2 + n0 + nn]),
                                             start=(c == 0), stop=(c == 3))
                        nc.vector.tensor_copy(_r(pT[:, m * P2 + n0: m * P2 + n0 + nn]),
                                              acc[:, 0:nn])

            # ---- Phase 3: q/k (transposed) and v (natural) projections ----
            with tc.tile_pool(name="sC", bufs=1) as sC:
                for wname in ("Wq", "Wk", "Wv"):
                    wsb = sC.tile([128, 2048], FP32, tag="wsb", bufs=2, name="wsb")
                    for c in range(4):
                        nc.sync.dma_start(out=_r(wsb[:, c * 512:(c + 1) * 512]),
                                          in_=_r(w_d[wname][c * 128:(c + 1) * 128, :]))
                    if wname in ("Wq", "Wk"):
                        for m in range(4):
                            for half in range(2):
                                acc = mmt()
                                for c in range(4):
                                    nc.tensor.matmul(
                                        acc,
                                        _r(wsb[:, c * 512 + m * 128: c * 512 + m * 128 + 128]),
                                        _r(xT[:, c * 1024 + half * 512: c * 1024 + (half + 1) * 512]),
                                        start=(c == 0), stop=(c == 3))
                                lo = m * 1024 + half * 512
                                if wname == "Wq":
                                    nc.vector.tensor_scalar_add(_r(qU[:, lo:lo + 512]), acc,
                                                                pbu_sb[:, m:m + 1])
                                    nc.vector.tensor_scalar_add(_r(qV[:, lo:lo + 512]), acc,
                                                                pbv_sb[:, m:m + 1])
                                else:
                                    nc.vector.tensor_scalar_add(_r(kT[:, lo:lo + 512]), acc,
                                                                bk_sb[:, m:m + 1])
                    else:
                        for tb in range(8):
                            acc = mmt()
                            for c in range(4):
                                nc.tensor.matmul(acc,
                                                 _r(xT[:, c * 1024 + tb * 128: c * 1024 + tb * 128 + 128]),
                                                 _r(wsb[:, c * 512:(c + 1) * 512]),
                                                 start=(c == 0), stop=False)
                            nc.tensor.matmul(acc, _r(ones[0:1, 0:128]), _r(bv_sb),
                                             start=False, stop=True)
                            nc.vector.tensor_copy(
                                v65[:, tb * 520:(tb + 1) * 520].rearrange("p (h e) -> p h e", e=65)[:, :, 0:64],
                                acc.rearrange("p (h e) -> p h e", e=64))
                for h in range(H):
                    nc.sync.dma_start(out=_r(wo_sb[0:64, h * 512:(h + 1) * 512]),
                                      in_=_r(w_d["Wo"][h * 64:(h + 1) * 64, :]))

            sX_cm.__exit__(None, None, None)

            with tc.tile_pool(name="work", bufs=1) as wk:
                # ---- Phase 4: pos scores -> exp -> banded skew write to DRAM ----
                for h in range(H):
                    ch, rh = h // 2, (h % 2) * 64
                    for tb in range(8):
                        t0 = tb * 128
                        b0 = 896 - t0
                        ep = wk.tile([128, 1152], FP16, tag="ep", bufs=3, name="ep")
                        for (o, nn) in ((0, 512), (512, 512), (1023, 128)):
                            acc = mmt()
                            nc.tensor.matmul(acc[:, 0:nn],
                                             _r(qV[rh:rh + 64, ch * 1024 + t0: ch * 1024 + t0 + 128]),
                                             _r(pT[rh:rh + 64, ch * P2 + b0 + o: ch * P2 + b0 + o + nn]),
                                             start=True, stop=True)
                            nc.scalar.activation(out=ep[:, o:o + nn], in_=acc[:, 0:nn],
                                                 func=EXP, bias=ebias, scale=SCALE)
                        off = t0 * P2 + b0
                        nc.sync.dma_start(
                            out=pd[h][off: off + 128 * P2].rearrange("(a b) -> a b", b=P2)[:, 0:BAND],
                            in_=ep[:, 0:BAND])

                # ---- Phase 5: content exp, skew-transposed readback, combine, AV, normalize ----
                for h in range(H):
                    ch, rh = h // 2, (h % 2) * 64
                    attn = wk.tile([128, 8192], FP16, tag="attn", bufs=1, name="attn")
                    for sb in range(8):
                        s0 = sb * 128
                        ecT = wk.tile([128, 1024], FP16, tag="ecT", bufs=3, name="ecT")
                        for half in range(2):
                            acc = mmt()
                            nc.tensor.matmul(acc,
                                             _r(kT[rh:rh + 64, ch * 1024 + s0: ch * 1024 + s0 + 128]),
                                             _r(qU[rh:rh + 64, ch * 1024 + half * 512: ch * 1024 + (half + 1) * 512]),
                                             start=True, stop=True)
                            nc.scalar.activation(out=ecT[:, half * 512:(half + 1) * 512], in_=acc,
                                                 func=EXP, bias=ebias, scale=SCALE)
                        epT = wk.tile([128, 1024], FP16, tag="epT", bufs=3, name="epT")
                        nc.sync.dma_start_transpose(
                            out=epT,
                            in_=pd[h][1023 + s0: 1023 + s0 + 1024 * 2046].rearrange(
                                "(a b) -> a b", b=2046)[:, 0:128])
                        nc.vector.tensor_tensor(attn[:, sb * 1024:(sb + 1) * 1024], ecT, epT, MUL)
                    avs = [psp.tile([65, 512], FP32, tag="av", bufs=2, name="av") for _ in range(2)]
                    for half in range(2):
                        for sb in range(8):
                            nc.tensor.matmul(avs[half],
                                             v65[:, sb * 520 + h * 65: sb * 520 + h * 65 + 65],
                                             attn[:, sb * 1024 + half * 512: sb * 1024 + (half + 1) * 512],
                                             start=(sb == 0), stop=(sb == 7))
                    rec = wk.tile([65, 1024], FP32, tag="rec", bufs=1, name="rec")
                    rbc = wk.tile([64, 1024], FP32, tag="rbc", bufs=1, name="rbc")
                    for half in range(2):
                        with nc.allow_low_precision(reason="f32r is fp32 bits"):
                            nc.vector.reciprocal(
                                out=_r(rec[64:65, half * 512:(half + 1) * 512]),
                                in_=avs[half][64:65, :])
                        bc = mmt()
                        nc.tensor.matmul(bc[0:64, :], _r(ones[64:65, 0:64]),
                                         _r(rec[64:65, half * 512:(half + 1) * 512]),
                                         start=True, stop=True)
                        nc.scalar.copy(out=rbc[:, half * 512:(half + 1) * 512], in_=bc[0:64, :])
                        nc.vector.tensor_tensor(
                            _r(outT[:, h * 1024 + half * 512: h * 1024 + (half + 1) * 512]),
                            avs[half][0:64, :],
                            rbc[:, half * 512:(half + 1) * 512], MUL)

                # ---- Phase 6: Wo projection + final transpose ----
                fins = [wk.tile([128, 512], FP32, tag=f"fin{i}", bufs=1, name=f"fin{i}")
                        for i in range(8)]
                for half in range(2):
                    for m in range(4):
                        acc = mmt()
                        for h in range(H):
                            nc.tensor.matmul(
                                acc,
                                _r(wo_sb[0:64, h * 512 + m * 128: h * 512 + m * 128 + 128]),
                                _r(outT[:, h * 1024 + half * 512: h * 1024 + (half + 1) * 512]),
                                start=(h == 0), stop=(h == 7))
                        ft = wk.tile([128, 512], FP32, tag="ft", bufs=2, name="ft")
                        nc.vector.tensor_scalar_add(ft, acc, bo_sb[:, m:m + 1])
                        for q4 in range(4):
                            tb = half * 4 + q4
                            ptr = mmt()
                            nc.tensor.transpose(ptr[:, 0:128], ft[:, q4 * 128:(q4 + 1) * 128], ident)
                            nc.scalar.copy(out=fins[tb][:, m * 128:(m + 1) * 128], in_=ptr[:, 0:128])

                for tb in range(8):
                    nc.sync.dma_start(out=out_d[tb * 128:(tb + 1) * 128, :], in_=fins[tb])

    nc.compile()
    return nc


_CACHE = {}
LAST = None


def kernel(**inputs):
    global LAST
    if "nc" not in _CACHE:
        _CACHE["nc"] = _build()
    nc = _CACHE["nc"]
    f32 = np.float32
    bq = np.asarray(inputs["bq"], f32)
    base = {
        "pos_enc": np.ascontiguousarray(np.asarray(inputs["pos_enc"], f32)[0]),
        "pbu": np.ascontiguousarray(np.asarray(inputs["pos_bias_u"], f32).reshape(-1) + bq),
        "pbv": np.ascontiguousarray(np.asarray(inputs["pos_bias_v"], f32).reshape(-1) + bq),
        "onesv": np.ones(128, f32),
    }
    for nm in ("Wq", "Wk", "bk", "Wv", "bv", "Wpos", "Wo", "bo"):
        base[nm] = np.ascontiguousarray(np.asarray(inputs[nm], f32))
    x = np.asarray(inputs["x"], f32)
    in_maps = [dict(base, x=np.ascontiguousarray(x[i])) for i in range(8)]
    trace = os.environ.get("KERNEL_TRACE", "0") == "1"
    res = bass_utils.run_bass_kernel_spmd(nc, in_maps, list(range(8)), trace=trace)
    LAST = res
    return np.stack([np.asarray(res.results[i]["out"], f32) for i in range(8)])


# revision 49
# speedup vs baseline: 1.0573x; 1.0573x over previous
import os
import sys

sys.path.insert(0, "/opt/trn_rl_repo")

import numpy as np

from concourse import mybir, bass, bacc, tile, bass_utils
from concourse.masks import make_identity

T = 1024
D = 512
H = 8
DH = 64
P2 = 2 * T - 1  # 2047
SCALE = 0.125
EB = -4.0  # multiplicative exp-split offset: exp(S*c+EB)*exp(S*p+EB) = e^{2EB} exp(S*(c+p))
BAND = 1151
PDSZ = T * P2 + 4096

FP32 = mybir.dt.float32
F32R = mybir.dt.float32r
FP16 = mybir.dt.float16
EXP = mybir.ActivationFunctionType.Exp
CPY = mybir.ActivationFunctionType.Copy
MUL = mybir.AluOpType.mult


def _r(ap):
    return ap.bitcast(F32R)


def _build():
    nc = bacc.Bacc()
    x_d = nc.declare_dram_parameter("x", (T, D), FP32, isOutput=False)
    pe_d = nc.declare_dram_parameter("pos_enc", (P2, D), FP32, isOutput=False)
    w_d = {}
    for nm in ("Wq", "Wk", "Wv", "Wpos", "Wo"):
        w_d[nm] = nc.declare_dram_parameter(nm, (D, D), FP32, isOutput=False)
    b_d = {}
    for nm in ("bk", "bv", "bo"):
        b_d[nm] = nc.declare_dram_parameter(nm, (D,), FP32, isOutput=False)
    pbu_d = nc.declare_dram_parameter("pbu", (D,), FP32, isOutput=False)
    pbv_d = nc.declare_dram_parameter("pbv", (D,), FP32, isOutput=False)
    ones_d = nc.declare_dram_parameter("onesv", (128,), FP32, isOutput=False)
    out_d = nc.declare_dram_parameter("out", (T, D), FP32, isOutput=True)

    with tile.TileContext(nc) as tc:
        with tc.tile_pool(name="persist", bufs=1) as pers, \
             tc.tile_pool(name="psum", bufs=1, space="PSUM") as psp, \
             tc.tile_pool(name="dram", bufs=1, space="DRAM") as drp:

            def mmt():
                return psp.tile([128, 512], FP32, tag="mm", bufs=6, name="mmt")

            ident = pers.tile([128, 128], FP32, tag="ident", name="ident")
            make_identity(nc, ident)
            identf = pers.tile([128, 128], FP32, tag="identf", name="identf")
            nc.vector.tensor_copy(_r(identf), ident)
            ones = pers.tile([128, 128], FP32, tag="ones", name="ones")
            nc.sync.dma_start(out=_r(ones[0:1, 0:128]),
                              in_=_r(ones_d[0:128].rearrange("(o f) -> o f", o=1)))
            nc.sync.dma_start(out=_r(ones[64:65, 0:64]),
                              in_=_r(ones_d[0:64].rearrange("(o f) -> o f", o=1)))
            ebias = pers.tile([128, 1], FP32, tag="ebias", name="ebias")
            nc.vector.memset(ebias, EB)

            bk_sb = pers.tile([128, 4], FP32, tag="bk", name="bk_sb")
            bo_sb = pers.tile([128, 4], FP32, tag="bo", name="bo_sb")
            bv_sb = pers.tile([1, 512], FP32, tag="bv", name="bv_sb")
            nc.sync.dma_start(out=_r(bv_sb[:, :]),
                              in_=_r(b_d["bv"][0:D].rearrange("(o f) -> o f", o=1)))
            for c in range(4):
                nc.sync.dma_start(out=bk_sb[:, c:c + 1],
                                  in_=b_d["bk"][c * 128:(c + 1) * 128].rearrange("(p o) -> p o", o=1))
                nc.sync.dma_start(out=bo_sb[:, c:c + 1],
                                  in_=b_d["bo"][c * 128:(c + 1) * 128].rearrange("(p o) -> p o", o=1))
            pbu_sb = pers.tile([128, 4], FP32, tag="pbu", name="pbu_sb")
            pbv_sb = pers.tile([128, 4], FP32, tag="pbv", name="pbv_sb")
            for c in range(4):
                nc.sync.dma_start(out=pbu_sb[:, c:c + 1],
                                  in_=pbu_d[c * 128:(c + 1) * 128].rearrange("(p o) -> p o", o=1))
                nc.sync.dma_start(out=pbv_sb[:, c:c + 1],
                                  in_=pbv_d[c * 128:(c + 1) * 128].rearrange("(p o) -> p o", o=1))

            # Persistent transposed activations: [p=feature%128, c*1024 + t]
            qU = pers.tile([128, 4096], FP32, tag="qU", name="qU")
            qV = pers.tile([128, 4096], FP32, tag="qV", name="qV")
            kT = pers.tile([128, 4096], FP32, tag="kT", name="kT")
            # v natural with ones column: [p=t%128, sb*520 + h*65 + e], e==64 -> 1.0
            v65 = pers.tile([128, 4160], FP16, tag="v65", name="v65")
            nc.vector.memset(v65.rearrange("p (b e) -> p b e", e=65)[:, :, 64:65], 1.0)
            # pT[p, c*2047 + n] = P[n, c*128+p],  P = pos_enc @ Wpos
            pT = pers.tile([128, 8188], FP32, tag="pT", name="pT")

            pd = [drp.tile([PDSZ], FP16, tag=f"pd{h}", name=f"pd{h}") for h in range(H)]

            sX_cm = tc.tile_pool(name="sX", bufs=1)
            sX = sX_cm.__enter__()
            xT = sX.tile([128, 4096], FP32, tag="xT", name="xT")
            sE_cm = tc.tile_pool(name="sE", bufs=1)
            sE = sE_cm.__enter__()
            sB_cm = tc.tile_pool(name="sB", bufs=1)
            sB = sB_cm.__enter__()

            # ---- Phase 1: xT via f32r matmul-transposes ----
            for tb in range(8):
                xin = sB.tile([128, 512], FP32, tag="xin", bufs=2, name="xin")
                nc.sync.dma_start(out=_r(xin), in_=_r(x_d[tb * 128:(tb + 1) * 128, :]))
                for c in range(4):
                    pt = mmt()
                    nc.tensor.matmul(pt[:, 0:128], _r(xin[:, c * 128:(c + 1) * 128]),
                                     _r(identf), start=True, stop=True)
                    nc.vector.tensor_copy(
                        _r(xT[:, c * 1024 + tb * 128: c * 1024 + tb * 128 + 128]),
                        pt[:, 0:128])

            # ---- Phase 2a: peT via f32r matmul-transposes ----
            peT = sB.tile([128, 8188], FP32, tag="peT", name="peT")
            wpos = sB.tile([128, 2048], FP32, tag="wpos", name="wpos")
            for c in range(4):
                nc.sync.dma_start(out=_r(wpos[:, c * 512:(c + 1) * 512]),
                                  in_=_r(w_d["Wpos"][c * 128:(c + 1) * 128, :]))
            for r in range(16):
                r0 = r * 128
                nr = min(128, P2 - r0)
                pin = sB.tile([128, 512], FP32, tag="pin", bufs=2, name="pin")
                if nr == 128:
                    nc.sync.dma_start(out=_r(pin), in_=_r(pe_d[r0:r0 + nr, :]))
                else:
                    nc.sync.dma_start(out=pin[0:nr, :], in_=pe_d[r0:r0 + nr, :])
                for c in range(4):
                    pt = mmt()
                    if nr == 128:
                        nc.tensor.matmul(pt[:, 0:128], _r(pin[:, c * 128:(c + 1) * 128]),
                                         _r(identf), start=True, stop=True)
                    else:
                        nc.tensor.transpose(pt[:, 0:nr], pin[0:nr, c * 128:(c + 1) * 128],
                                            ident[0:nr, 0:nr])
                    nc.vector.tensor_copy(_r(peT[:, c * P2 + r0: c * P2 + r0 + nr]),
                                          pt[:, 0:nr])

            wq_sb = sB.tile([128, 2048], FP32, tag="wq", name="wq_sb")
            for c in range(4):
                nc.sync.dma_start(out=_r(wq_sb[:, c * 512:(c + 1) * 512]),
                                  in_=_r(w_d["Wq"][c * 128:(c + 1) * 128, :]))

            # ---- Interleaved: pT chunk m -> qU/qV chunk m -> phase4 heads 2m,2m+1 ----
            for m in range(4):
                for (n0, nn) in ((0, 512), (512, 512), (1024, 512), (1535, 512)):
                    acc = mmt()
                    for c in range(4):
                        nc.tensor.matmul(acc[:, 0:nn],
                                         _r(wpos[:, c * 512 + m * 128: c * 512 + m * 128 + 128]),
                                         _r(peT[:, c * P2 + n0: c * P2 + n0 + nn]),
                                         start=(c == 0), stop=(c == 3))
                    nc.vector.tensor_copy(_r(pT[:, m * P2 + n0: m * P2 + n0 + nn]),
                                          acc[:, 0:nn])
                for half in range(2):
                    acc = mmt()
                    for c in range(4):
                        nc.tensor.matmul(
                            acc,
                            _r(wq_sb[:, c * 512 + m * 128: c * 512 + m * 128 + 128]),
                            _r(xT[:, c * 1024 + half * 512: c * 1024 + (half + 1) * 512]),
                            start=(c == 0), stop=(c == 3))
                    lo = m * 1024 + half * 512
                    nc.vector.tensor_scalar_add(_r(qU[:, lo:lo + 512]), acc,
                                                pbu_sb[:, m:m + 1])
                    nc.vector.tensor_scalar_add(_r(qV[:, lo:lo + 512]), acc,
                                                pbv_sb[:, m:m + 1])
                # Phase 4 for the two heads living in feature chunk m
                for h in (2 * m, 2 * m + 1):
                    ch, rh = h // 2, (h % 2) * 64
                    for tb in range(8):
                        t0 = tb * 128
                        b0 = 896 - t0
                        ep = sE.tile([128, 1152], FP16, tag="ep", bufs=3, name="ep")
                        for (o, nn) in ((0, 512), (512, 512), (1023, 128)):
                            acc = mmt()
                            nc.tensor.matmul(acc[:, 0:nn],
                                             _r(qV[rh:rh + 64, ch * 1024 + t0: ch * 1024 + t0 + 128]),
                                             _r(pT[rh:rh + 64, ch * P2 + b0 + o: ch * P2 + b0 + o + nn]),
                                             start=True, stop=True)
                            nc.scalar.activation(out=ep[:, o:o + nn], in_=acc[:, 0:nn],
                                                 func=EXP, bias=ebias, scale=SCALE)
                        off = t0 * P2 + b0
                        nc.sync.dma_start(
                            out=pd[h][off: off + 128 * P2].rearrange("(a b) -> a b", b=P2)[:, 0:BAND],
                            in_=ep[:, 0:BAND])

            sB_cm.__exit__(None, None, None)

            # ---- Phase 3b: k (transposed) and v (natural) projections ----
            sC_cm = tc.tile_pool(name="sC", bufs=1)
            sC = sC_cm.__enter__()
            for wname in ("Wk", "Wv"):
                wsb = sC.tile([128, 2048], FP32, tag="wsb", bufs=2, name="wsb")
                for c in range(4):
                    nc.sync.dma_start(out=_r(wsb[:, c * 512:(c + 1) * 512]),
                                      in_=_r(w_d[wname][c * 128:(c + 1) * 128, :]))
                if wname == "Wk":
                    for m in range(4):
                        for half in range(2):
                            acc = mmt()
                            for c in range(4):
                                nc.tensor.matmul(
                                    acc,
                                    _r(wsb[:, c * 512 + m * 128: c * 512 + m * 128 + 128]),
                                    _r(xT[:, c * 1024 + half * 512: c * 1024 + (half + 1) * 512]),
                                    start=(c == 0), stop=(c == 3))
                            lo = m * 1024 + half * 512
                            nc.vector.tensor_scalar_add(_r(kT[:, lo:lo + 512]), acc,
                                                        bk_sb[:, m:m + 1])
                else:
                    for tb in range(8):
                        acc = mmt()
                        for c in range(4):
                            nc.tensor.matmul(acc,
                                             _r(xT[:, c * 1024 + tb * 128: c * 1024 + tb * 128 + 128]),
                                             _r(wsb[:, c * 512:(c + 1) * 512]),
                                             start=(c == 0), stop=False)
                        nc.tensor.matmul(acc, _r(ones[0:1, 0:128]), _r(bv_sb),
                                         start=False, stop=True)
                        nc.vector.tensor_copy(
                            v65[:, tb * 520:(tb + 1) * 520].rearrange("p (h e) -> p h e", e=65)[:, :, 0:64],
                            acc.rearrange("p (h e) -> p h e", e=64))
            sC_cm.__exit__(None, None, None)
            sE_cm.__exit__(None, None, None)
            sX_cm.__exit__(None, None, None)

            with tc.tile_pool(name="work", bufs=1) as wk:
                # wo_sb[p, h*512 + dout] = Wo[h*64 + p, dout]  (all heads at base partition 0)
                wo_sb = wk.tile([64, 4096], FP32, tag="wo", name="wo_sb")
                for h in range(H):
                    nc.sync.dma_start(out=_r(wo_sb[0:64, h * 512:(h + 1) * 512]),
                                      in_=_r(w_d["Wo"][h * 64:(h + 1) * 64, :]))
                outT = wk.tile([64, 8192], FP32, tag="outT", name="outT")

                # ---- Phase 5: content exp, skew-transposed readback, combine, AV, normalize ----
                for h in range(H):
                    ch, rh = h // 2, (h % 2) * 64
                    attn = wk.tile([128, 8192], FP16, tag="attn", bufs=1, name="attn")
                    for sb in range(8):
                        s0 = sb * 128
                        ecT = wk.tile([128, 1024], FP16, tag="ecT", bufs=3, name="ecT")
                        for half in range(2):
                            acc = mmt()
                            nc.tensor.matmul(acc,
                                             _r(kT[rh:rh + 64, ch * 1024 + s0: ch * 1024 + s0 + 128]),
                                             _r(qU[rh:rh + 64, ch * 1024 + half * 512: ch * 1024 + (half + 1) * 512]),
                                             start=True, stop=True)
                            nc.scalar.activation(out=ecT[:, half * 512:(half + 1) * 512], in_=acc,
                                                 func=EXP, bias=ebias, scale=SCALE)
                        epT = wk.tile([128, 1024], FP16, tag="epT", bufs=3, name="epT")
                        nc.sync.dma_start_transpose(
                            out=epT,
                            in_=pd[h][1023 + s0: 1023 + s0 + 1024 * 2046].rearrange(
                                "(a b) -> a b", b=2046)[:, 0:128])
                        nc.vector.tensor_tensor(attn[:, sb * 1024:(sb + 1) * 1024], ecT, epT, MUL)
                    avs = [psp.tile([65, 512], FP32, tag="av", bufs=2, name="av") for _ in range(2)]
                    for half in range(2):
                        for sb in range(8):
                            nc.tensor.matmul(avs[half],
                                             v65[:, sb * 520 + h * 65: sb * 520 + h * 65 + 65],
                                             attn[:, sb * 1024 + half * 512: sb * 1024 + (half + 1) * 512],
                                             start=(sb == 0), stop=(sb == 7))
                    rec = wk.tile([65, 1024], FP32, tag="rec", bufs=1, name="rec")
                    rbc = wk.tile([64, 1024], FP32, tag="rbc", bufs=1, name="rbc")
                    for half in range(2):
                        with nc.allow_low_precision(reason="f32r is fp32 bits"):
                            nc.vector.reciprocal(
                                out=_r(rec[64:65, half * 512:(half + 1) * 512]),
                                in_=avs[half][64:65, :])
                        bc = mmt()
                        nc.tensor.matmul(bc[0:64, :], _r(ones[64:65, 0:64]),
                                         _r(rec[64:65, half * 512:(half + 1) * 512]),
                                         start=True, stop=True)
                        nc.gpsimd.tensor_copy(rbc[:, half * 512:(half + 1) * 512], bc[0:64, :])
                        nc.vector.tensor_tensor(
                            _r(outT[:, h * 1024 + half * 512: h * 1024 + (half + 1) * 512]),
                            avs[half][0:64, :],
                            rbc[:, half * 512:(half + 1) * 512], MUL)

                # ---- Phase 6: Wo projection + final transpose ----
                fins = [wk.tile([128, 512], FP32, tag=f"fin{i}", bufs=1, name=f"fin{i}")
                        for i in range(8)]
                for half in range(2):
                    for m in range(4):
                        acc = mmt()
                        for h in range(H):
                            nc.tensor.matmul(
                                acc,
                                _r(wo_sb[0:64, h * 512 + m * 128: h * 512 + m * 128 + 128]),
                                _r(outT[:, h * 1024 + half * 512: h * 1024 + (half + 1) * 512]),
                                start=(h == 0), stop=(h == 7))
                        ft = wk.tile([128, 512], FP32, tag="ft", bufs=2, name="ft")
                        nc.vector.tensor_scalar_add(_r(ft), acc, bo_sb[:, m:m + 1])
                        for q4 in range(4):
                            tb = half * 4 + q4
                            ptr = mmt()
                            nc.tensor.matmul(ptr[:, 0:128], _r(ft[:, q4 * 128:(q4 + 1) * 128]),
                                             _r(identf), start=True, stop=True)
                            nc.gpsimd.tensor_copy(fins[tb][:, m * 128:(m + 1) * 128], ptr[:, 0:128])

                for tb in range(8):
                    nc.sync.dma_start(out=out_d[tb * 128:(tb + 1) * 128, :], in_=fins[tb])

    nc.compile()
    return nc


_CACHE = {}
LAST = None


def kernel(**inputs):
    global LAST
    if "nc" not in _CACHE:
        _CACHE["nc"] = _build()
    nc = _CACHE["nc"]
    f32 = np.float32
    bq = np.asarray(inputs["bq"], f32)
    base = {
        "pos_enc": np.ascontiguousarray(np.asarray(inputs["pos_enc"], f32)[0]),
        "pbu": np.ascontiguousarray(np.asarray(inputs["pos_bias_u"], f32).reshape(-1) + bq),
        "pbv": np.ascontiguousarray(np.asarray(inputs["pos_bias_v"], f32).reshape(-1) + bq),
        "onesv": np.ones(128, f32),
    }
    for nm in ("Wq", "Wk", "bk", "Wv", "bv", "Wpos", "Wo", "bo"):
        base[nm] = np.ascontiguousarray(np.asarray(inputs[nm], f32))
    x = np.asarray(inputs["x"], f32)
    in_maps = [dict(base, x=np.ascontiguousarray(x[i])) for i in range(8)]
    trace = os.environ.get("KERNEL_TRACE", "0") == "1"
    res = bass_utils.run_bass_kernel_spmd(nc, in_maps, list(range(8)), trace=trace)
    LAST = res
    return np.stack([np.asarray(res.results[i]["out"], f32) for i in range(8)])
